# revision 38
# baseline (speedup 1.0000x reference)
"""Trainium2 Bass kernel for nn_CMDPEncoder (VQ codebook quantize + random
batch-mix dequantize + DP noise).

Reference semantics:
    dots = einsum('bsd,vd->bsv', base, codebook)
    qi   = argmin_v(csq[v] - 2*dots)                  # [B,S]
    codes[b,s,j] = qi[rand_idx[b,s,j], s]
    out  = mean_j codebook[codes] + 0.1*noise

Sharding: split the sequence dim S across the 8 cores (64 positions each).
rand_idx mixing crosses only the batch dim at fixed s, so with S-sharding
every core's mixing is fully local (no collectives).  Tokens are laid out
s-major (t = s_local*16 + b) so each 128-token tile holds 8 complete
s-groups of 16 batches, and the mix becomes a block-diagonal [128,128]
matmul with host-precomputed weights (counts/4 from rand_idx).

Pipeline per 128-token tile:
  - bf16 scoring matmuls (6 chunk passes + 1 fp16 csq-Dekker bias pass per
    512-code vblock) into PSUM; ACT drains PSUM->SBUF at f32.
  - DVE MAX8 (top-8 values) + FIND_INDEX8 (their indices) on the f32
    scores; f32 needles make duplicate-value aliasing a non-issue.
  - K=2 rescue: gpsimd indirect-gathers the top-2 codebook rows (fp32,
    csq in col 768), DVE STT computes exact dj = x.c - csq/2, strict-gt
    picks the winner; winner INDEX = ci0 + flip*(ci1-ci0) (host-verified
    zero argmax flips on this dataset, margin ~7x fp32-STT noise).
  - winner row for the mix is indirect-gathered from a bf16 codebook
    table (no ACT select copies at all).
  - mix: block-diagonal [128,128] bf16 matmul; noise added via
    DMA-accumulate (prestaged + DVE add for the last 2 tiles to shorten
    the tail); out stored bf16, upcast on host.
Startup: a short dummy-matmul prewarm covers the PE's slow p-state
(427ns/pass -> 216ns/pass), then phase-1 scores v-block 0 across ALL
tiles (x16 tiles are small and stream fast) so scoring starts ~7us
earlier with no codebook-stream pacing stalls; v0 results stage in SBUF
and are copied into each tile's score row. The last tile pipelines its
output halves through separate DMA queues to shorten the final chain.
"""

import sys

for p in ("/opt/trn_rl_repo",):
    if p not in sys.path:
        sys.path.insert(0, p)

import numpy as np

import concourse.bacc as bacc
import concourse.bass as bass
import concourse.mybir as mybir
import concourse.tile as tile
from concourse.bass_utils import run_bass_kernel_spmd

B, S, D, V = 16, 512, 768, 4096
N_CORES = 8
SS = S // N_CORES            # 64 sequence positions per core
T = SS * B                   # 1024 tokens per core, t = s_local*16 + b
TT = T // 128                # 8 token tiles per core
KC = D // 128                # 6 contraction chunks of 128
NV = V // 512                # 8 v-blocks of 512 codes
DP_EPSILON = 0.1
CSQ_CENTER = 768.0
DE = 776                     # padded cbe row: 768 cb + 1 csq + 7 pad

F32 = mybir.dt.float32
F16 = mybir.dt.float16
BF16 = mybir.dt.bfloat16
U32 = mybir.dt.uint32
I32 = mybir.dt.int32
ALU = mybir.AluOpType

_CACHED = {}


def _build_nc():
    nc = bacc.Bacc("TRN2", target_bir_lowering=False, debug=False,
                   num_devices=N_CORES)

    x16_d = nc.dram_tensor("x16", [128, TT * KC * 128], BF16,
                           kind="ExternalInput")
    cb16_d = nc.dram_tensor("cb16", [128, NV * KC * 512], BF16,
                            kind="ExternalInput")
    csqL_d = nc.dram_tensor("csqL", [2, T], F16, kind="ExternalInput")
    csqR_d = nc.dram_tensor("csqR", [2, V], F16, kind="ExternalInput")
    cbe_d = nc.dram_tensor("cbe", [V, DE], F32, kind="ExternalInput")
    cby_d = nc.dram_tensor("cby", [V, D], BF16, kind="ExternalInput")
    xn_d = nc.dram_tensor("xn", [128, TT * DE], F32, kind="ExternalInput")
    w_d = nc.dram_tensor("w", [128, TT * 128], BF16, kind="ExternalInput")
    noise_d = nc.dram_tensor("noise", [T, D], BF16, kind="ExternalInput")
    out_d = nc.dram_tensor("out", [T, D], BF16, kind="ExternalOutput")

    XTW = KC * 128
    VBW = KC * 512

    with tile.TileContext(nc) as tc:
        with (
            tc.tile_pool(name="big", bufs=1) as big,
            tc.tile_pool(name="sc", bufs=2) as sc_pool,
            tc.tile_pool(name="work", bufs=2) as work,
            tc.tile_pool(name="yp", bufs=4) as yp,
            tc.tile_pool(name="io", bufs=3) as io,
            tc.tile_pool(name="ps_s", bufs=6, space="PSUM") as ps_s,
            tc.tile_pool(name="ps_m", bufs=1, space="PSUM") as ps_m,
        ):
            # ---- PE prewarm: dummy matmuls on memset data so the PE is
            # out of its slow p-state before real scoring starts --------
            wrm = big.tile([128, 512], BF16, tag="wrm")
            nc.gpsimd.memset(wrm[:], 0.0)
            psw = ps_s.tile([128, 512], F32, tag="ps_score", name="ps_warm")
            for _ in range(8):
                nc.tensor.matmul(psw[:], wrm[:, 0:128], wrm[:],
                                 start=True, stop=True)

            # ---- persistent input staging ------------------------------
            csql = big.tile([2, T], F16)
            csqr = big.tile([2, V], F16)
            nc.sync.dma_start(csql[:], csqL_d.ap())
            nc.sync.dma_start(csqr[:], csqR_d.ap())

            x16_t, cb16_v, xn_t = [], [], []
            # phase-1 scores v-block 0 across ALL tiles, so load every x16
            # tile right after cb v0 (they are small); remaining v-blocks
            # stream while phase-1 runs and never outpace the PE
            tl = big.tile([128, KC, 128], BF16, tag="x16_0")
            nc.sync.dma_start(tl[:], x16_d.ap()[:, 0:XTW])
            x16_t.append(tl)
            tl = big.tile([128, KC, 512], BF16, tag="cb16_0")
            nc.sync.dma_start(tl[:], cb16_d.ap()[:, 0:VBW])
            cb16_v.append(tl)
            for t in range(1, TT):
                tl = big.tile([128, KC, 128], BF16, tag=f"x16_{t}")
                nc.sync.dma_start(tl[:], x16_d.ap()[:, t * XTW:(t + 1) * XTW])
                x16_t.append(tl)
            for v in range(1, NV):
                tl = big.tile([128, KC, 512], BF16, tag=f"cb16_{v}")
                nc.sync.dma_start(tl[:], cb16_d.ap()[:, v * VBW:(v + 1) * VBW])
                cb16_v.append(tl)
                if v == 3:
                    # xn tile 0 early: the rescore of tile 0 needs it
                    tl = big.tile([128, DE], F32, tag="xn_0")
                    nc.sync.dma_start(tl[:], xn_d.ap()[:, 0:DE])
                    xn_t.append(tl)
            for t in range(1, TT):
                tl = big.tile([128, DE], F32, tag=f"xn_{t}")
                nc.sync.dma_start(tl[:], xn_d.ap()[:, t * DE:(t + 1) * DE])
                xn_t.append(tl)
            w16 = big.tile([128, TT * 128], BF16)
            nc.sync.dma_start(w16[:], w_d.ap())
            # last two tiles' noise prestaged: their add runs on DVE
            # directly from PSUM, shortening the end-of-run chain
            nz_tail = {}
            for t in (TT - 2, TT - 1):
                tl = big.tile([128, D], BF16, tag=f"nz_{t}")
                nc.sync.dma_start(tl[:],
                                  noise_d.ap()[t * 128:(t + 1) * 128, :])
                nz_tail[t] = tl

            # ---- phase 1: v-block 0 for every tile, staged to SBUF -----
            v0st = big.tile([128, TT, 512], F32)
            for t in range(TT):
                tsl = slice(t * 128, (t + 1) * 128)
                ps = ps_s.tile([128, 512], F32, tag="ps_score",
                               name=f"p1_{t}")
                for k in range(KC):
                    nc.tensor.matmul(ps[:], x16_t[t][:, k, :],
                                     cb16_v[0][:, k, :],
                                     start=(k == 0), stop=False)
                nc.tensor.matmul(ps[:], csql[:, tsl], csqr[:, 0:512],
                                 start=False, stop=True)
                nc.scalar.copy(out=v0st[:, t, :], in_=ps[:])

            def emit_scoring(t):
                tsl = slice(t * 128, (t + 1) * 128)
                scores = sc_pool.tile([128, V], F32, tag="scores")
                nc.scalar.copy(out=scores[:, 0:512], in_=v0st[:, t, :])
                for grp in ((1, 2, 3, 4), (5, 6, 7)):
                    pss = []
                    for vi, v in enumerate(grp):
                        ps = ps_s.tile([128, 512], F32, tag="ps_score",
                                       name=f"ps_{t}_{v}")
                        pss.append(ps)
                    for k in range(KC):
                        for vi, v in enumerate(grp):
                            nc.tensor.matmul(pss[vi][:], x16_t[t][:, k, :],
                                             cb16_v[v][:, k, :],
                                             start=(k == 0), stop=False)
                    for vi, v in enumerate(grp):
                        vsl = slice(v * 512, (v + 1) * 512)
                        nc.tensor.matmul(pss[vi][:], csql[:, tsl],
                                         csqr[:, vsl], start=False, stop=True)
                        nc.scalar.copy(out=scores[:, vsl], in_=pss[vi][:])
                return scores

            def emit_scan_a(t, scores):
                """top-8 values+indices -> launch top-2 fp32 row gather."""
                m8 = work.tile([128, 8], F32, tag="m8")
                nc.vector.max(m8[:], scores[:])
                gidx = work.tile([128, 8], U32, tag="gidx")
                nc.vector.max_index(gidx[:], m8[:], scores[:])
                # gather offsets straight from the u32 indices (values
                # <4096, bit-identical to i32) - no cast on the critical
                # path; the f32 copy for the winner arithmetic runs while
                # the gathers are in flight
                g0 = work.tile([128, DE], F32, tag="g0")
                nc.gpsimd.indirect_dma_start(
                    out=g0[:], out_offset=None, in_=cbe_d.ap(),
                    in_offset=bass.IndirectOffsetOnAxis(ap=gidx[:, 0:1],
                                                        axis=0))
                g1 = work.tile([128, DE], F32, tag="g1")
                nc.gpsimd.indirect_dma_start(
                    out=g1[:], out_offset=None, in_=cbe_d.ap(),
                    in_offset=bass.IndirectOffsetOnAxis(ap=gidx[:, 1:2],
                                                        axis=0))
                y0 = yp.tile([128, D], BF16, tag="y0")
                nc.gpsimd.indirect_dma_start(
                    out=y0[:], out_offset=None, in_=cby_d.ap(),
                    in_offset=bass.IndirectOffsetOnAxis(ap=gidx[:, 0:1],
                                                        axis=0))
                y1 = yp.tile([128, D], BF16, tag="y1")
                nc.gpsimd.indirect_dma_start(
                    out=y1[:], out_offset=None, in_=cby_d.ap(),
                    in_offset=bass.IndirectOffsetOnAxis(ap=gidx[:, 1:2],
                                                        axis=0))
                return g0, g1, y0, y1

            def emit_scan_b(t, g0, g1, y0, y1):
                """exact rescore of the 2 gathered rows -> winner index ->
                bf16 winner row via indirect gather.

                The rescore dot runs over all DE=776 gathered columns: the
                xn tile carries -0.5 at col 768 (csq slot) and 0 in the pad,
                so accum = x.g - csq/2 and the strict-gt compare needs no
                extra bias ops."""
                dj = work.tile([128, 2], F32, tag="dj")
                t0 = work.tile([128, DE], F32, tag="rs_tmp0")
                nc.vector.scalar_tensor_tensor(
                    out=t0[:], in0=xn_t[t][:], scalar=1.0, in1=g0[:],
                    op0=ALU.bypass, op1=ALU.mult, accum_out=dj[:, 0:1])
                t1 = work.tile([128, DE], F32, tag="rs_tmp1")
                nc.vector.scalar_tensor_tensor(
                    out=t1[:], in0=xn_t[t][:], scalar=1.0, in1=g1[:],
                    op0=ALU.bypass, op1=ALU.mult, accum_out=dj[:, 1:2])

                flip = work.tile([128, 1], F32, tag="flip")
                nc.vector.tensor_tensor(out=flip[:], in0=dj[:, 1:2],
                                        in1=dj[:, 0:1], op=ALU.is_gt)
                yd = work.tile([128, D], BF16, tag="yd")
                nc.vector.tensor_tensor(out=yd[:], in0=y1[:], in1=y0[:],
                                        op=ALU.subtract)
                y = yp.tile([128, D], BF16, tag="y")
                nc.vector.scalar_tensor_tensor(
                    out=y[:], in0=yd[:], scalar=flip[:, 0:1], in1=y0[:],
                    op0=ALU.mult, op1=ALU.add)
                return y

            def emit_output(t, y):
                tsl = slice(t * 128, (t + 1) * 128)
                pm = ps_m.tile([128, D], F32, tag="pm")
                nc.tensor.matmul(pm[:, 0:512], w16[:, tsl], y[:, 0:512],
                                 start=True, stop=True)
                nc.tensor.matmul(pm[:, 512:D], w16[:, tsl], y[:, 512:D],
                                 start=True, stop=True)
                ob = io.tile([128, D], BF16, tag="out")
                if t == TT - 1:
                    # last tile: pipeline the two mix halves through add and
                    # DMA on separate queues so nothing waits for the full
                    # row and the final HBM write acks overlap
                    nc.vector.tensor_tensor(out=ob[:, 0:512],
                                            in0=pm[:, 0:512],
                                            in1=nz_tail[t][:, 0:512],
                                            op=ALU.add)
                    nc.scalar.dma_start(out_d.ap()[tsl, 0:512],
                                        ob[:, 0:512])
                    nc.vector.tensor_tensor(out=ob[:, 512:D],
                                            in0=pm[:, 512:D],
                                            in1=nz_tail[t][:, 512:D],
                                            op=ALU.add)
                    nc.sync.dma_start(out_d.ap()[tsl, 512:D],
                                      ob[:, 512:D])
                    return
                if t in nz_tail:
                    nc.vector.tensor_tensor(out=ob[:], in0=pm[:],
                                            in1=nz_tail[t][:],
                                            op=ALU.add)
                else:
                    nc.scalar.copy(out=ob[:], in_=pm[:])
                    nc.gpsimd.dma_start(out=ob[:], in_=noise_d.ap()[tsl, :],
                                        accum_op=ALU.add)
                nc.sync.dma_start(out_d.ap()[tsl, :], ob[:])

            # software pipeline: scan_b(t-1) runs while scan_a(t)'s gather
            # is in flight; mix and output trail by PIPE tiles.
            PIPE = 3
            gq = []
            pending = []
            for t in range(TT):
                scores = emit_scoring(t)
                ga = emit_scan_a(t, scores)
                gq.append((t, ga))
                if len(gq) > 1:
                    tb, gb = gq.pop(0)
                    pending.append((tb, emit_scan_b(tb, *gb)))
                if len(pending) > PIPE:
                    emit_output(*pending.pop(0))
            while gq:
                tb, gb = gq.pop(0)
                pending.append((tb, emit_scan_b(tb, *gb)))
            for item in pending:
                emit_output(*item)

    nc.compile()
    return nc


def _prep_inputs(base_embeddings, codebook, rand_idx, noise):
    """Build the 8 per-core input maps (all host-side numpy)."""
    import ml_dtypes
    bf = ml_dtypes.bfloat16

    x = np.ascontiguousarray(base_embeddings, dtype=np.float32)
    cb = np.ascontiguousarray(codebook, dtype=np.float32)
    ridx = np.asarray(rand_idx)
    nz = np.asarray(noise, dtype=np.float32)

    csq = (cb * cb).sum(-1, dtype=np.float32)              # [V]
    cbe = np.zeros((V, DE), np.float32)
    cbe[:, :D] = cb
    cbe[:, D] = csq
    cby = cb.astype(bf)
    csqc = (csq - CSQ_CENTER).astype(np.float32)
    r1 = csqc.astype(np.float16)
    r2 = (csqc - r1.astype(np.float32)).astype(np.float16)
    csqR = np.ascontiguousarray(np.stack([r1, r2]))        # [2, V] fp16
    csqL = np.full((2, T), -1.0, np.float16)

    # cb16: [V, 768] -> [128, NV, KC, 512]
    cb16 = np.ascontiguousarray(
        cb.reshape(NV, 512, KC, 128).transpose(3, 0, 2, 1)
        .reshape(128, NV * KC * 512)).astype(bf)

    shared = {"cbe": cbe, "cby": cby, "csqL": csqL, "csqR": csqR,
              "cb16": cb16}

    in_maps = []
    for c in range(N_CORES):
        ssl = slice(c * SS, (c + 1) * SS)
        xc = x[:, ssl, :].transpose(1, 0, 2).reshape(T, D)  # s-major tokens
        x2 = 2.0 * xc
        x16 = np.ascontiguousarray(
            x2.reshape(TT, 128, KC, 128).transpose(3, 0, 2, 1)
            .reshape(128, TT * KC * 128)).astype(bf)
        xne = np.zeros((T, DE), np.float32)
        xne[:, :D] = xc
        xne[:, D] = -0.5                                   # csq slot weight
        xn = np.ascontiguousarray(
            xne.reshape(TT, 128, DE).transpose(1, 0, 2).reshape(128, TT * DE))
        nzc = np.ascontiguousarray(
            DP_EPSILON * nz[:, ssl, :].transpose(1, 0, 2).reshape(T, D)
        ).astype(bf)
        rc = ridx[:, ssl, :]                               # [B, SS, K]
        wm = np.zeros((TT, 128, 128), np.float32)
        for tt in range(TT):
            for gges in range(8):
                s_local = tt * 8 + gges
                r = rc[:, s_local, :]                      # [B, K] in [0,B)
                cnt = np.zeros((B, B), np.float32)         # [dst=b, src]
                for bdst in range(B):
                    np.add.at(cnt[bdst], r[bdst], 1.0)
                wm[tt, gges * 16:(gges + 1) * 16,
                   gges * 16:(gges + 1) * 16] = cnt.T / 4.0
        wm_t = np.ascontiguousarray(
            wm.transpose(1, 0, 2).reshape(128, TT * 128)).astype(bf)
        m = {"x16": x16, "xn": xn, "w": wm_t, "noise": nzc, **shared}
        in_maps.append(m)
    return in_maps


def kernel(base_embeddings, codebook, rand_idx, noise, _results_out=None):
    if "nc" not in _CACHED:
        _CACHED["nc"] = _build_nc()
    nc = _CACHED["nc"]
    in_maps = _prep_inputs(base_embeddings, codebook, rand_idx, noise)
    res = run_bass_kernel_spmd(nc, in_maps, list(range(N_CORES)))
    if _results_out is not None:
        _results_out.append(res)
    outs = []
    for c in range(N_CORES):
        oc = res.results[c]["out"].astype(np.float32)
        oc = oc.reshape(SS, B, D).transpose(1, 0, 2)
        outs.append(oc)
    return np.ascontiguousarray(np.concatenate(outs, axis=1))


# revision 39
# speedup vs baseline: 1.1978x; 1.1978x over previous
"""Trainium2 Bass kernel for nn_CMDPEncoder (VQ codebook quantize + random
batch-mix dequantize + DP noise).

Reference semantics:
    dots = einsum('bsd,vd->bsv', base, codebook)
    qi   = argmin_v(csq[v] - 2*dots)                  # [B,S]
    codes[b,s,j] = qi[rand_idx[b,s,j], s]
    out  = mean_j codebook[codes] + 0.1*noise

Sharding: split the sequence dim S across the 8 cores (64 positions each).
rand_idx mixing crosses only the batch dim at fixed s, so with S-sharding
every core's mixing is fully local (no collectives).  Tokens are laid out
s-major (t = s_local*16 + b) so each 128-token tile holds 8 complete
s-groups of 16 batches, and the mix becomes a block-diagonal [128,128]
matmul with host-precomputed weights (counts/4 from rand_idx).

Pipeline per 128-token tile:
  - bf16 scoring matmuls (6 chunk passes + 1 fp16 csq-Dekker bias pass per
    512-code vblock) into PSUM; ACT drains PSUM->SBUF at f32.
  - DVE MAX8 (top-8 values) + FIND_INDEX8 (their indices) on the f32
    scores; f32 needles make duplicate-value aliasing a non-issue.
  - K=2 rescue: gpsimd indirect-gathers the top-2 codebook rows (fp32,
    csq in col 768), DVE STT computes exact dj = x.c - csq/2, strict-gt
    picks the winner; winner INDEX = ci0 + flip*(ci1-ci0) (host-verified
    zero argmax flips on this dataset, margin ~7x fp32-STT noise).
  - winner row for the mix is indirect-gathered from a bf16 codebook
    table (no ACT select copies at all).
  - mix: block-diagonal [128,128] bf16 matmul; noise added via
    DMA-accumulate (prestaged + DVE add for the last 2 tiles to shorten
    the tail); out stored bf16, upcast on host.
Startup: a short dummy-matmul prewarm covers the PE's slow p-state
(427ns/pass -> 216ns/pass), then phase-1 scores v-block 0 across ALL
tiles (x16 tiles are small and stream fast) so scoring starts ~7us
earlier with no codebook-stream pacing stalls; v0 results stage in SBUF
and are copied into each tile's score row. The last tile pipelines its
output halves through separate DMA queues to shorten the final chain.
"""

import sys

for p in ("/opt/trn_rl_repo",):
    if p not in sys.path:
        sys.path.insert(0, p)

import numpy as np

import concourse.bacc as bacc
import concourse.bass as bass
import concourse.mybir as mybir
import concourse.tile as tile
from concourse.bass_utils import run_bass_kernel_spmd

B, S, D, V = 16, 512, 768, 4096
N_CORES = 8
SS = S // N_CORES            # 64 sequence positions per core
T = SS * B                   # 1024 tokens per core, t = s_local*16 + b
TT = T // 128                # 8 token tiles per core
KC = D // 128                # 6 contraction chunks of 128
NV = V // 512                # 8 v-blocks of 512 codes
DP_EPSILON = 0.1
CSQ_CENTER = 768.0
DE = 776                     # padded cbe row: 768 cb + 1 csq + 7 pad

F32 = mybir.dt.float32
F16 = mybir.dt.float16
BF16 = mybir.dt.bfloat16
U32 = mybir.dt.uint32
I32 = mybir.dt.int32
ALU = mybir.AluOpType

_CACHED = {}


def _build_nc():
    nc = bacc.Bacc("TRN2", target_bir_lowering=False, debug=False,
                   num_devices=N_CORES)

    x16_d = nc.dram_tensor("x16", [128, TT * KC * 128], BF16,
                           kind="ExternalInput")
    cb16_d = nc.dram_tensor("cb16", [128, NV * KC * 512], BF16,
                            kind="ExternalInput")
    csqL_d = nc.dram_tensor("csqL", [2, T], F16, kind="ExternalInput")
    csqR_d = nc.dram_tensor("csqR", [2, V], F16, kind="ExternalInput")
    cbe_d = nc.dram_tensor("cbe", [V, DE], F32, kind="ExternalInput")
    cby_d = nc.dram_tensor("cby", [V, D], BF16, kind="ExternalInput")
    xn_d = nc.dram_tensor("xn", [128, TT * DE], F32, kind="ExternalInput")
    w_d = nc.dram_tensor("w", [128, TT * 128], BF16, kind="ExternalInput")
    noise_d = nc.dram_tensor("noise", [T, D], BF16, kind="ExternalInput")
    out_d = nc.dram_tensor("out", [T, D], BF16, kind="ExternalOutput")

    XTW = KC * 128
    VBW = KC * 512

    with tile.TileContext(nc) as tc:
        with (
            tc.tile_pool(name="big", bufs=1) as big,
            tc.tile_pool(name="sc", bufs=2) as sc_pool,
            tc.tile_pool(name="work", bufs=2) as work,
            tc.tile_pool(name="yp", bufs=4) as yp,
            tc.tile_pool(name="io", bufs=3) as io,
            tc.tile_pool(name="ps_s", bufs=6, space="PSUM") as ps_s,
            tc.tile_pool(name="ps_m", bufs=1, space="PSUM") as ps_m,
        ):
            # ---- PE prewarm: dummy matmuls on memset data so the PE is
            # out of its slow p-state before real scoring starts --------
            wrm = big.tile([128, 512], BF16, tag="wrm")
            nc.gpsimd.memset(wrm[:], 0.0)
            psw = ps_s.tile([128, 512], F32, tag="ps_score", name="ps_warm")
            for _ in range(8):
                nc.tensor.matmul(psw[:], wrm[:, 0:128], wrm[:],
                                 start=True, stop=True)

            # ---- persistent input staging ------------------------------
            csql = big.tile([2, T], F16)
            csqr = big.tile([2, V], F16)
            nc.sync.dma_start(csql[:], csqL_d.ap())
            nc.sync.dma_start(csqr[:], csqR_d.ap())

            x16_t, cb16_v, xn_t = [], [], []
            # phase-1 scores v-block 0 across ALL tiles, so load every x16
            # tile right after cb v0 (they are small); remaining v-blocks
            # stream while phase-1 runs and never outpace the PE
            tl = big.tile([128, KC, 128], BF16, tag="x16_0")
            nc.sync.dma_start(tl[:], x16_d.ap()[:, 0:XTW])
            x16_t.append(tl)
            tl = big.tile([128, KC, 512], BF16, tag="cb16_0")
            nc.sync.dma_start(tl[:], cb16_d.ap()[:, 0:VBW])
            cb16_v.append(tl)
            for t in range(1, TT):
                tl = big.tile([128, KC, 128], BF16, tag=f"x16_{t}")
                nc.sync.dma_start(tl[:], x16_d.ap()[:, t * XTW:(t + 1) * XTW])
                x16_t.append(tl)
            for v in range(1, NV):
                tl = big.tile([128, KC, 512], BF16, tag=f"cb16_{v}")
                nc.sync.dma_start(tl[:], cb16_d.ap()[:, v * VBW:(v + 1) * VBW])
                cb16_v.append(tl)
                if v == 3:
                    # xn tile 0 early: the rescore of tile 0 needs it
                    tl = big.tile([128, DE], F32, tag="xn_0")
                    nc.sync.dma_start(tl[:], xn_d.ap()[:, 0:DE])
                    xn_t.append(tl)
            for t in range(1, TT):
                tl = big.tile([128, DE], F32, tag=f"xn_{t}")
                nc.sync.dma_start(tl[:], xn_d.ap()[:, t * DE:(t + 1) * DE])
                xn_t.append(tl)
            w16 = big.tile([128, TT * 128], BF16)
            nc.sync.dma_start(w16[:], w_d.ap())
            # last two tiles' noise prestaged: their add runs on DVE
            # directly from PSUM, shortening the end-of-run chain
            nz_tail = {}
            for t in (TT - 2, TT - 1):
                tl = big.tile([128, D], BF16, tag=f"nz_{t}")
                nc.sync.dma_start(tl[:],
                                  noise_d.ap()[t * 128:(t + 1) * 128, :])
                nz_tail[t] = tl

            # ---- phase 1: v-block 0 for every tile, staged to SBUF -----
            v0st = big.tile([128, TT, 512], F32)
            for t in range(TT):
                tsl = slice(t * 128, (t + 1) * 128)
                ps = ps_s.tile([128, 512], F32, tag="ps_score",
                               name=f"p1_{t}")
                for k in range(KC):
                    nc.tensor.matmul(ps[:], x16_t[t][:, k, :],
                                     cb16_v[0][:, k, :],
                                     start=(k == 0), stop=False)
                nc.tensor.matmul(ps[:], csql[:, tsl], csqr[:, 0:512],
                                 start=False, stop=True)
                nc.scalar.copy(out=v0st[:, t, :], in_=ps[:])

            def emit_scoring(t):
                tsl = slice(t * 128, (t + 1) * 128)
                scores = sc_pool.tile([128, V], F32, tag="scores")
                nc.scalar.copy(out=scores[:, 0:512], in_=v0st[:, t, :])
                for grp in ((1, 2, 3, 4), (5, 6, 7)):
                    pss = []
                    for vi, v in enumerate(grp):
                        ps = ps_s.tile([128, 512], F32, tag="ps_score",
                                       name=f"ps_{t}_{v}")
                        pss.append(ps)
                    for k in range(KC):
                        for vi, v in enumerate(grp):
                            nc.tensor.matmul(pss[vi][:], x16_t[t][:, k, :],
                                             cb16_v[v][:, k, :],
                                             start=(k == 0), stop=False)
                    for vi, v in enumerate(grp):
                        vsl = slice(v * 512, (v + 1) * 512)
                        nc.tensor.matmul(pss[vi][:], csql[:, tsl],
                                         csqr[:, vsl], start=False, stop=True)
                        nc.scalar.copy(out=scores[:, vsl], in_=pss[vi][:])
                return scores

            def emit_scan_a(t, scores):
                """top-8 values+indices -> launch top-2 fp32 row gather."""
                m8 = work.tile([128, 8], F32, tag="m8")
                nc.vector.max(m8[:], scores[:])
                gidx = work.tile([128, 8], U32, tag="gidx")
                nc.vector.max_index(gidx[:], m8[:], scores[:])
                # gather offsets straight from the u32 indices (values
                # <4096, bit-identical to i32) - no cast on the critical
                # path; the f32 copy for the winner arithmetic runs while
                # the gathers are in flight
                g0 = work.tile([128, DE], F32, tag="g0")
                nc.gpsimd.indirect_dma_start(
                    out=g0[:], out_offset=None, in_=cbe_d.ap(),
                    in_offset=bass.IndirectOffsetOnAxis(ap=gidx[:, 0:1],
                                                        axis=0))
                g1 = work.tile([128, DE], F32, tag="g1")
                nc.gpsimd.indirect_dma_start(
                    out=g1[:], out_offset=None, in_=cbe_d.ap(),
                    in_offset=bass.IndirectOffsetOnAxis(ap=gidx[:, 1:2],
                                                        axis=0))
                cif = work.tile([128, 2], F32, tag="cif")
                nc.gpsimd.tensor_copy(cif[:], gidx[:, 0:2])
                return g0, g1, cif

            def emit_scan_b(t, g0, g1, cif):
                """exact rescore of the 2 gathered rows -> winner index ->
                bf16 winner row via indirect gather.

                The rescore dot runs over all DE=776 gathered columns: the
                xn tile carries -0.5 at col 768 (csq slot) and 0 in the pad,
                so accum = x.g - csq/2 and the strict-gt compare needs no
                extra bias ops."""
                dj = work.tile([128, 2], F32, tag="dj")
                t0 = work.tile([128, DE], F32, tag="rs_tmp0")
                nc.vector.scalar_tensor_tensor(
                    out=t0[:], in0=xn_t[t][:], scalar=1.0, in1=g0[:],
                    op0=ALU.bypass, op1=ALU.mult, accum_out=dj[:, 0:1])
                t1 = work.tile([128, DE], F32, tag="rs_tmp1")
                nc.vector.scalar_tensor_tensor(
                    out=t1[:], in0=xn_t[t][:], scalar=1.0, in1=g1[:],
                    op0=ALU.bypass, op1=ALU.mult, accum_out=dj[:, 1:2])

                dd = work.tile([128, 1], F32, tag="dd")
                nc.vector.tensor_tensor(out=dd[:], in0=cif[:, 1:2],
                                        in1=cif[:, 0:1], op=ALU.subtract)
                fd = work.tile([128, 1], F32, tag="fd")
                nc.vector.scalar_tensor_tensor(
                    out=fd[:], in0=dj[:, 1:2], scalar=dj[:, 0:1], in1=dd[:],
                    op0=ALU.is_gt, op1=ALU.mult)
                iwf = work.tile([128, 1], F32, tag="iwf")
                nc.vector.tensor_tensor(out=iwf[:], in0=cif[:, 0:1],
                                        in1=fd[:], op=ALU.add)
                iw = work.tile([128, 1], I32, tag="iw")
                nc.gpsimd.tensor_copy(iw[:], iwf[:])

                y = yp.tile([128, D], BF16, tag="y")
                nc.gpsimd.indirect_dma_start(
                    out=y[:], out_offset=None, in_=cby_d.ap(),
                    in_offset=bass.IndirectOffsetOnAxis(ap=iw[:, 0:1],
                                                        axis=0))
                return y

            def emit_output(t, y):
                tsl = slice(t * 128, (t + 1) * 128)
                pm = ps_m.tile([128, D], F32, tag="pm")
                nc.tensor.matmul(pm[:, 0:512], w16[:, tsl], y[:, 0:512],
                                 start=True, stop=True)
                nc.tensor.matmul(pm[:, 512:D], w16[:, tsl], y[:, 512:D],
                                 start=True, stop=True)
                ob = io.tile([128, D], BF16, tag="out")
                if t == TT - 1:
                    # last tile: pipeline the two mix halves through add and
                    # DMA on separate queues so nothing waits for the full
                    # row and the final HBM write acks overlap
                    nc.vector.tensor_tensor(out=ob[:, 0:512],
                                            in0=pm[:, 0:512],
                                            in1=nz_tail[t][:, 0:512],
                                            op=ALU.add)
                    nc.scalar.dma_start(out_d.ap()[tsl, 0:512],
                                        ob[:, 0:512])
                    nc.vector.tensor_tensor(out=ob[:, 512:D],
                                            in0=pm[:, 512:D],
                                            in1=nz_tail[t][:, 512:D],
                                            op=ALU.add)
                    nc.sync.dma_start(out_d.ap()[tsl, 512:D],
                                      ob[:, 512:D])
                    return
                if t in nz_tail:
                    nc.vector.tensor_tensor(out=ob[:], in0=pm[:],
                                            in1=nz_tail[t][:],
                                            op=ALU.add)
                else:
                    nc.scalar.copy(out=ob[:], in_=pm[:])
                    nc.gpsimd.dma_start(out=ob[:], in_=noise_d.ap()[tsl, :],
                                        accum_op=ALU.add)
                nc.sync.dma_start(out_d.ap()[tsl, :], ob[:])

            # software pipeline: scan_b(t-1) runs while scan_a(t)'s gather
            # is in flight; mix and output trail by PIPE tiles.
            PIPE = 3
            gq = []
            pending = []
            for t in range(TT):
                scores = emit_scoring(t)
                ga = emit_scan_a(t, scores)
                gq.append((t, ga))
                if len(gq) > 1:
                    tb, gb = gq.pop(0)
                    pending.append((tb, emit_scan_b(tb, *gb)))
                if len(pending) > PIPE:
                    emit_output(*pending.pop(0))
            while gq:
                tb, gb = gq.pop(0)
                pending.append((tb, emit_scan_b(tb, *gb)))
            for item in pending:
                emit_output(*item)

    nc.compile()
    return nc


def _prep_inputs(base_embeddings, codebook, rand_idx, noise):
    """Build the 8 per-core input maps (all host-side numpy)."""
    import ml_dtypes
    bf = ml_dtypes.bfloat16

    x = np.ascontiguousarray(base_embeddings, dtype=np.float32)
    cb = np.ascontiguousarray(codebook, dtype=np.float32)
    ridx = np.asarray(rand_idx)
    nz = np.asarray(noise, dtype=np.float32)

    csq = (cb * cb).sum(-1, dtype=np.float32)              # [V]
    cbe = np.zeros((V, DE), np.float32)
    cbe[:, :D] = cb
    cbe[:, D] = csq
    cby = cb.astype(bf)
    csqc = (csq - CSQ_CENTER).astype(np.float32)
    r1 = csqc.astype(np.float16)
    r2 = (csqc - r1.astype(np.float32)).astype(np.float16)
    csqR = np.ascontiguousarray(np.stack([r1, r2]))        # [2, V] fp16
    csqL = np.full((2, T), -1.0, np.float16)

    # cb16: [V, 768] -> [128, NV, KC, 512]
    cb16 = np.ascontiguousarray(
        cb.reshape(NV, 512, KC, 128).transpose(3, 0, 2, 1)
        .reshape(128, NV * KC * 512)).astype(bf)

    shared = {"cbe": cbe, "cby": cby, "csqL": csqL, "csqR": csqR,
              "cb16": cb16}

    in_maps = []
    for c in range(N_CORES):
        ssl = slice(c * SS, (c + 1) * SS)
        xc = x[:, ssl, :].transpose(1, 0, 2).reshape(T, D)  # s-major tokens
        x2 = 2.0 * xc
        x16 = np.ascontiguousarray(
            x2.reshape(TT, 128, KC, 128).transpose(3, 0, 2, 1)
            .reshape(128, TT * KC * 128)).astype(bf)
        xne = np.zeros((T, DE), np.float32)
        xne[:, :D] = xc
        xne[:, D] = -0.5                                   # csq slot weight
        xn = np.ascontiguousarray(
            xne.reshape(TT, 128, DE).transpose(1, 0, 2).reshape(128, TT * DE))
        nzc = np.ascontiguousarray(
            DP_EPSILON * nz[:, ssl, :].transpose(1, 0, 2).reshape(T, D)
        ).astype(bf)
        rc = ridx[:, ssl, :]                               # [B, SS, K]
        wm = np.zeros((TT, 128, 128), np.float32)
        for tt in range(TT):
            for gges in range(8):
                s_local = tt * 8 + gges
                r = rc[:, s_local, :]                      # [B, K] in [0,B)
                cnt = np.zeros((B, B), np.float32)         # [dst=b, src]
                for bdst in range(B):
                    np.add.at(cnt[bdst], r[bdst], 1.0)
                wm[tt, gges * 16:(gges + 1) * 16,
                   gges * 16:(gges + 1) * 16] = cnt.T / 4.0
        wm_t = np.ascontiguousarray(
            wm.transpose(1, 0, 2).reshape(128, TT * 128)).astype(bf)
        m = {"x16": x16, "xn": xn, "w": wm_t, "noise": nzc, **shared}
        in_maps.append(m)
    return in_maps


def kernel(base_embeddings, codebook, rand_idx, noise, _results_out=None):
    if "nc" not in _CACHED:
        _CACHED["nc"] = _build_nc()
    nc = _CACHED["nc"]
    in_maps = _prep_inputs(base_embeddings, codebook, rand_idx, noise)
    res = run_bass_kernel_spmd(nc, in_maps, list(range(N_CORES)))
    if _results_out is not None:
        _results_out.append(res)
    outs = []
    for c in range(N_CORES):
        oc = res.results[c]["out"].astype(np.float32)
        oc = oc.reshape(SS, B, D).transpose(1, 0, 2)
        outs.append(oc)
    return np.ascontiguousarray(np.concatenate(outs, axis=1))
